# revision 16
# baseline (speedup 1.0000x reference)
"""Distributed causal multi-head attention block on 8 TRN2 NeuronCores.

Tensor-parallel over heads (2 heads/core), batch-packed attention:
  - host: pre-cast to bf16, pre-transpose x -> xT [C, B*T], shard W_attn
    columns by head pair, permute W_proj rows to the AllToAll delivery order.
  - phase 1 streams xT k-chunks in interleaved batch order (r = 0,4,1,5,...)
    computing qT/kT into per-head tiles qk_h [128 = b*64+d, 2, T] (q slot 0,
    k slot 1) via partition-shifting DVE copies, with the qkv bias folded
    into the matmul as a K=1 ones-row term; v goes to vext (natural layout,
    extra ones column per head for softmax row sums).
  - attention per head h, batch-PACKED: per 512-query block and 128-key
    chunk, the two batches' s = kT.T@qT run as two concurrent K=64 row-tiled
    matmuls (tile_position (0,0)/(64,0)) into separate PSUM banks, one Exp
    activation covers both, triangle mask on VectorE for diagonal chunks,
    av^T accumulated per batch with the ones column giving row sums free.
  - normalize: approx-reciprocal (DVE), broadcast across the 64 head dims
    via DRAM bounce for h0 units and via GpSimd partition_broadcast for h1
    units (keeps DMA latency and shared completion-ring coupling off the
    last-unit -> AllToAll(h1) trigger path), scale on DVE, stage into the
    per-head AllToAll.  h0's attention is emission-interleaved with phase 1
    so exp work starts at ~25us; h0's AllToAll overlaps h1's attention; the
    h0 half of the projection overlaps h1's AllToAll.  The warmup AllToAll
    is issued before any other DMA so the first-collective ncfw latency is
    absorbed as early as possible.
  - projection: per-core 512-row t-shard x full (row-permuted) W_proj,
    1-bank PSUM units, per-128-row output DMA.
"""

import numpy as np
import ml_dtypes

import concourse.bass as bass
import concourse.mybir as mybir
import concourse.tile as tile
from concourse import bacc
from concourse.bass_utils import run_bass_kernel_spmd

P = 128
B, T, C = 2, 2048, 1024
H, D = 16, 64
NCORES = 8
HPC = H // NCORES          # heads per core = 2
BT = B * T                 # 4096
TSH = BT // NCORES         # 512 rows per core shard
KC = C // P                # 8 contraction chunks
NBLK = BT // TSH           # 8 t-blocks of 512
QB = T // TSH              # 4 query blocks of 512 per batch
CH = T // P                # 16 key chunks of 128 per batch
F32 = mybir.dt.float32
BF16 = mybir.dt.bfloat16
SCALE = 1.0 / 8.0          # 1/sqrt(D)


def build_nc(with_qk_bias=True):
    nc = bacc.Bacc(None, target_bir_lowering=False)

    xT = nc.dram_tensor("xT", [C, BT], BF16, kind="ExternalInput")
    w_qk = nc.dram_tensor("w_qk", [C, 2 * P], BF16, kind="ExternalInput")
    w_v = nc.dram_tensor("w_v", [C, P], BF16, kind="ExternalInput")
    b_qk = nc.dram_tensor("b_qk", [2 * P], F32, kind="ExternalInput")
    b_v = nc.dram_tensor("b_v", [P], F32, kind="ExternalInput")
    w_pr = nc.dram_tensor("w_proj", [C, C], BF16, kind="ExternalInput")
    b_pr = nc.dram_tensor("b_proj", [C], F32, kind="ExternalInput")
    maskm = nc.dram_tensor("mask", [P, P], BF16, kind="ExternalInput")
    out = nc.dram_tensor("out", [TSH, C], F32, kind="ExternalOutput")

    with tile.TileContext(nc) as tc:
        with (
            tc.tile_pool(name="consts", bufs=1) as consts,
            tc.tile_pool(name="persist", bufs=1) as persist,
            tc.tile_pool(name="xtg", bufs=3) as xtg_pool,
            tc.tile_pool(name="pt", bufs=4) as pt_pool,
            tc.tile_pool(name="avs", bufs=2) as avs_pool,
            tc.tile_pool(name="rec", bufs=2) as rec_pool,
            tc.tile_pool(name="ps_s", bufs=2, space="PSUM") as ps_s,
            tc.tile_pool(name="ps_av", bufs=1, space="PSUM") as ps_av,
            tc.tile_pool(name="pp", bufs=2, space="PSUM") as pp,
            tc.tile_pool(name="dram", bufs=1, space="DRAM") as dram,
            tc.tile_pool(name="dram_rec", bufs=4, space="DRAM") as dram_rec,
        ):
            # tiny AllToAll to absorb the first-collective ncfw warmup latency
            warm_sb = consts.tile([1, 2 * NCORES], BF16)
            nc.vector.memset(warm_sb[:], 0.0)
            warm_in = dram.tile([NCORES, 2], BF16, name="warm_in")
            warm_out = dram.tile([NCORES, 2], BF16, name="warm_out")
            nc.sync.dma_start(
                warm_in.rearrange("(o r) m -> o (r m)", o=1), warm_sb[:]
            )
            nc.gpsimd.collective_compute(
                "AllToAll",
                mybir.AluOpType.bypass,
                ins=[warm_in.opt()],
                outs=[warm_out.opt()],
                replica_groups=[list(range(NCORES))],
            )

            # ---- first xT block + small constants first (critical path) ----
            xT_blocked = xT.ap().rearrange(
                "(kh kc p) (r t) -> r kh p kc t", p=P, r=NBLK, kh=2
            )
            # the small qkv weight load goes first - it and the first pair of
            # xT k-chunks are all the first matmul needs
            wqk_sb = consts.tile([P, KC, 2 * P], BF16)
            nc.sync.dma_start(wqk_sb[:], w_qk.ap().rearrange("(kc p) m -> p kc m", p=P))
            xtg_first = xtg_pool.tile([P, KC, TSH], BF16, name="xtg_first")
            for q in range(4):
                nc.sync.dma_start(
                    xtg_first[:, 2 * q:2 * q + 2, :],
                    xT_blocked[0, q // 2][:, 2 * (q % 2):2 * (q % 2) + 2, :],
                )
            wv_sb = consts.tile([P, KC, P], BF16)
            nc.sync.dma_start(wv_sb[:], w_v.ap().rearrange("(kc p) m -> p kc m", p=P))
            bqk_f32 = consts.tile([1, 2 * P], F32)
            nc.sync.dma_start(bqk_f32[:], b_qk.ap().rearrange("(o m) -> o m", o=1))
            bqk_row = consts.tile([1, 2 * P], BF16)
            nc.vector.tensor_copy(bqk_row[:], bqk_f32[:])
            ones_row = consts.tile([1, TSH], BF16)
            nc.vector.memset(ones_row[:], 1.0)
            bv_sb = consts.tile([P, P], F32)
            nc.sync.dma_start(
                bv_sb[:],
                b_v.ap().rearrange("(o m) -> o m", o=1).to_broadcast((P, P)),
            )
            mask_sb = consts.tile([P, P], BF16)
            nc.sync.dma_start(mask_sb[:], maskm.ap())
            mask_bc = mask_sb[:].rearrange("p (o m) -> p o m", o=1).to_broadcast(
                (P, 2, P)
            )


            # ---- persistent activation tiles ----
            # qk_h[h]: [b*64+d, slot(q=0,k=1), t-in-batch]
            qk_h = [
                persist.tile([P, 2, T], BF16, name=f"qk_h{h}") for h in range(HPC)
            ]
            # vext: [tk-part, global chunk, 130]: 0:64 v_h0, 64 ones,
            # 65:129 v_h1, 129 ones
            vext = persist.tile([P, BT // P, 130], BF16)
            nc.vector.memset(vext[:, :, 64], 1.0)
            nc.vector.memset(vext[:, :, 129], 1.0)

            a2a_in = [
                dram.tile([NCORES * D, TSH], BF16, name=f"a2a_in_{h}")
                for h in range(HPC)
            ]
            a2a_out = [
                dram.tile([NCORES * D, TSH], BF16, name=f"a2a_out_{h}")
                for h in range(HPC)
            ]

            # ---- phase 1 r-block: qT/kT (b-major partitions) + v ----
            # The qk matmuls (N=512, matmul-port-bound) and v matmuls
            # (stationary xT chunks, LDWEIGHTS-port-bound) are interleaved so
            # both PE ports stay saturated: 4 qk matmuls are spread into each
            # of the 4 v accumulation units.
            def emit_p1_block(r, xtg_r):
                b, ts = r // QB, (r % QB) * TSH
                ps = ps_s.tile([P, 2, TSH], F32, name="ps_qk", tag="s")
                for mt in range(TSH // P):
                    m, half = mt // 2, mt % 2
                    psv = pp.tile([P, TSH], F32, name="ps_v", tag="p")
                    for kc in range(KC):
                        if kc % 2 == 0:
                            qkc = half * (KC // 2) + kc // 2
                            nc.tensor.matmul(
                                ps[:, m, :],
                                lhsT=wqk_sb[:, qkc, m * P:(m + 1) * P],
                                rhs=xtg_r[:, qkc, :],
                                start=(qkc == 0),
                                stop=(not with_qk_bias and qkc == KC - 1),
                            )
                        nc.tensor.matmul(
                            psv[:, 0:P],
                            lhsT=xtg_r[:, kc, mt * P:(mt + 1) * P],
                            rhs=wv_sb[:, kc, :],
                            start=(kc == 0),
                            stop=(kc == KC - 1),
                        )
                    ch = b * CH + (r % QB) * (TSH // P) + mt
                    for h in range(HPC):
                        nc.vector.tensor_tensor(
                            vext[:, ch, h * 65:h * 65 + 64],
                            psv[:, h * D:(h + 1) * D],
                            bv_sb[:, h * D:(h + 1) * D],
                            mybir.AluOpType.add,
                        )
                if with_qk_bias:
                    for m in range(2):
                        # bias as a K=1 ones-row term closes the accumulation
                        nc.tensor.matmul(
                            ps[:, m, :],
                            lhsT=bqk_row[0:1, m * P:(m + 1) * P],
                            rhs=ones_row[:],
                            start=False,
                            stop=True,
                        )
                for h in range(HPC):
                    # partition-shifting cast copy into the b-major quadrant
                    nc.vector.tensor_copy(
                        qk_h[h][b * D:(b + 1) * D, :, ts:ts + TSH],
                        ps[h * D:(h + 1) * D, :, :],
                    )

            # ---- attention for (head h, 512-query block qb), batches packed ----
            def emit_attention_qb(h, qb):
                nch = (TSH // P) * (qb + 1)
                ps_o = ps_av.tile([65, 2, TSH], F32, name="ps_av", tag="av")
                for c in range(nch):
                    lo = max(0, (c - (TSH // P) * qb) * P)
                    ps = ps_s.tile([P, 2, TSH], F32, name="ps_sc", tag="s")
                    for b in range(2):
                        nc.tensor.matmul(
                            ps[:, b, lo:TSH],
                            lhsT=qk_h[h][b * D:(b + 1) * D, 1, c * P:(c + 1) * P],
                            rhs=qk_h[h][b * D:(b + 1) * D, 0,
                                        qb * TSH + lo:(qb + 1) * TSH],
                            start=True, stop=True,
                            tile_position=(b * D, 0),
                        )
                    pt = pt_pool.tile([P, 2, TSH], BF16)
                    nc.scalar.activation(
                        pt[:, :, lo:TSH], ps[:, :, lo:TSH],
                        mybir.ActivationFunctionType.Exp,
                        scale=SCALE,
                    )
                    if c >= (TSH // P) * qb:
                        # diagonal chunk: triangle mask on both batches at once
                        nc.vector.tensor_tensor(
                            pt[:, :, lo:lo + P], pt[:, :, lo:lo + P],
                            mask_bc,
                            mybir.AluOpType.mult,
                        )
                    for b in range(2):
                        nc.tensor.matmul(
                            ps_o[:, b, lo:TSH],
                            lhsT=vext[:, b * CH + c, h * 65:h * 65 + 65],
                            rhs=pt[:, b, lo:TSH],
                            start=(c == 0), stop=(c == nch - 1),
                        )
                # normalize: approx recip of the row sums, DRAM-bounce
                # broadcast across the 64 head dims, then scale and stage
                den = rec_pool.tile([1, 2, TSH], F32, name="den")
                nc.vector.tensor_copy(den[:], ps_o[64:65, :, :])
                av_f = avs_pool.tile([D, 2, TSH], F32, name="av_f")
                nc.vector.tensor_copy(av_f[:], ps_o[0:64, :, :])
                rec = rec_pool.tile([1, 2, TSH], F32, name="rec")
                nc.vector.reciprocal_approx_fast(rec[:], den[:])
                rec_rep = rec_pool.tile([D, 2, TSH], F32, name="rec_rep")
                if qb == 3:
                    # qb3 units feed straight into an AllToAll: broadcast on GpSimd
                    # (SBUF->SBUF) instead of a DRAM bounce, keeping DMA
                    # latency and collective-ring coupling off the
                    # last-unit -> A2A(h1) trigger path.
                    nc.gpsimd.partition_broadcast(rec_rep[:], rec[:])
                else:
                    rec_dram = dram_rec.tile([1, 2 * TSH], F32, name="rec_dram")
                    nc.sync.dma_start(
                        rec_dram[:], rec[:].rearrange("p s t -> p (s t)")
                    )
                    nc.sync.dma_start(
                        rec_rep[:].rearrange("p s t -> p (s t)"),
                        rec_dram[0:1, :].to_broadcast((D, 2 * TSH)),
                    )
                att_n = avs_pool.tile([D, 2, TSH], BF16, name="att_n")
                nc.vector.tensor_tensor(
                    att_n[:], av_f[:], rec_rep[:],
                    mybir.AluOpType.mult,
                )
                for b in range(2):
                    g = b * QB + qb
                    nc.sync.dma_start(
                        a2a_in[h][g * D:(g + 1) * D, :], att_n[:, b, :]
                    )

            # ---- phase 1 + h0 attention, emission-interleaved ----
            wpr_sb = consts.tile([P, KC, C], BF16)
            bpr_sb = consts.tile([P, C], F32)
            r_order = [0, QB, 1, QB + 1, 2, QB + 2, 3, QB + 3]
            for idx, r in enumerate(r_order):
                if r == 0:
                    xtg_r = xtg_first
                else:
                    xtg_r = xtg_pool.tile([P, KC, TSH], BF16)
                    nc.sync.dma_start(xtg_r[:, 0:KC // 2, :], xT_blocked[r, 0])
                    nc.sync.dma_start(xtg_r[:, KC // 2:KC, :], xT_blocked[r, 1])
                emit_p1_block(r, xtg_r)
                if idx % 2 == 1 and idx < 7:
                    # both heads' qb-unit deps exist after pair (qb, 4+qb);
                    # emitting both fills the ScalarE gap while the next
                    # block pair streams in.  qb=3 is held back to sequence
                    # the tail h0qb3 -> A2A(h0) -> h1qb3 -> A2A(h1).
                    emit_attention_qb(0, idx // 2)
                    emit_attention_qb(1, idx // 2)
                if idx == 3:
                    # proj weights issued mid-stream: after the xT blocks they
                    # would delay, well before the AllToAlls they'd contend with
                    nc.sync.dma_start(
                        wpr_sb[:], w_pr.ap().rearrange("(kc p) m -> p kc m", p=P)
                    )
                    nc.sync.dma_start(
                        bpr_sb[:],
                        b_pr.ap().rearrange("(o m) -> o m", o=1).to_broadcast((P, C)),
                    )
            emit_attention_qb(0, 3)
            nc.gpsimd.collective_compute(
                "AllToAll",
                mybir.AluOpType.bypass,
                ins=[a2a_in[0].opt()],
                outs=[a2a_out[0].opt()],
                replica_groups=[list(range(NCORES))],
            )
            emit_attention_qb(1, 3)
            nc.gpsimd.collective_compute(
                "AllToAll",
                mybir.AluOpType.bypass,
                ins=[a2a_in[1].opt()],
                outs=[a2a_out[1].opt()],
                replica_groups=[list(range(NCORES))],
            )

            # ---- phase 4: output projection for my t-shard ----
            # A2A h out rows: [src-rank r x 64] = channels (r, h); W_proj rows
            # were host-permuted to this order: kc chunk i<4 from h0, i>=4 h1.
            att_sb = [
                persist.tile([P, KC // 2, TSH], BF16, name=f"att_sb_{h}")
                for h in range(HPC)
            ]
            out_sb = persist.tile([P, TSH // P, C], F32)
            out_blocked = out.ap().rearrange("(mt p) c -> mt p c", p=P)
            for h in range(HPC):
                a2a_view = a2a_out[h].rearrange("(kc p) t -> kc p t", p=P)
                for kc in range(KC // 2):
                    nc.sync.dma_start(att_sb[h][:, kc, :], a2a_view[kc])
                for mt in range(TSH // P):
                    for nb in range(C // TSH):
                        psu = pp.tile([P, TSH], F32, name="ps_pr", tag="p")
                        for kc in range(KC // 2):
                            nc.tensor.matmul(
                                psu[:],
                                lhsT=att_sb[h][:, kc, mt * P:(mt + 1) * P],
                                rhs=wpr_sb[:, h * (KC // 2) + kc,
                                           nb * TSH:(nb + 1) * TSH],
                                start=(kc == 0),
                                stop=(kc == KC // 2 - 1),
                            )
                        other = (
                            bpr_sb[:, nb * TSH:(nb + 1) * TSH] if h == 0
                            else out_sb[:, mt, nb * TSH:(nb + 1) * TSH]
                        )
                        nc.vector.tensor_tensor(
                            out_sb[:, mt, nb * TSH:(nb + 1) * TSH],
                            psu[:],
                            other,
                            mybir.AluOpType.add,
                        )
                    if h == HPC - 1:
                        nc.sync.dma_start(out_blocked[mt], out_sb[:, mt, :])
    nc.finalize()
    return nc


_NC_CACHE = {}


def _get_nc(with_qk_bias=True):
    if with_qk_bias not in _NC_CACHE:
        _NC_CACHE[with_qk_bias] = build_nc(with_qk_bias)
    return _NC_CACHE[with_qk_bias]


def make_in_maps(x, W_attn, b_attn, W_proj, b_proj):
    bf = ml_dtypes.bfloat16
    x_flat = np.asarray(x, np.float32).reshape(BT, C)
    xT_bf = np.ascontiguousarray(x_flat.T).astype(bf)
    W_attn = np.asarray(W_attn, np.float32)
    b_attn = np.asarray(b_attn, np.float32)
    b_proj = np.asarray(b_proj, np.float32)
    # permute W_proj rows to the split-A2A delivery order:
    # [r0h0 | r1h0 | ... | r7h0 | r0h1 | ... | r7h1]
    perm = np.concatenate(
        [np.arange(r * P + h * D, r * P + (h + 1) * D)
         for h in range(HPC) for r in range(NCORES)]
    )
    W_proj_bf = np.ascontiguousarray(
        np.asarray(W_proj, np.float32)[perm]
    ).astype(bf)
    mask = (np.arange(P)[None, :] >= np.arange(P)[:, None]).astype(bf)

    in_maps = []
    for i in range(NCORES):
        cs = slice(i * P, (i + 1) * P)
        w_qk = np.concatenate(
            [W_attn[:, 0:C][:, cs], W_attn[:, C:2 * C][:, cs]], axis=1
        ).astype(bf)
        b_qk = np.concatenate(
            [b_attn[0:C][cs], b_attn[C:2 * C][cs]]
        ).astype(np.float32)
        in_maps.append({
            "xT": xT_bf,
            "w_qk": np.ascontiguousarray(w_qk),
            "w_v": np.ascontiguousarray(W_attn[:, 2 * C:3 * C][:, cs]).astype(bf),
            "b_qk": np.ascontiguousarray(b_qk),
            "b_v": np.ascontiguousarray(b_attn[2 * C:3 * C][cs]).astype(np.float32),
            "w_proj": W_proj_bf,
            "b_proj": b_proj,
            "mask": mask,
        })
    return in_maps


def kernel(x, W_attn, b_attn, W_proj, b_proj):
    nc = _get_nc(bool(np.any(np.asarray(b_attn))))
    in_maps = make_in_maps(x, W_attn, b_attn, W_proj, b_proj)
    res = run_bass_kernel_spmd(nc, in_maps, core_ids=list(range(NCORES)))
    shards = [np.asarray(res.results[i]["out"], np.float32) for i in range(NCORES)]
    return np.concatenate(shards, axis=0).reshape(B, T, C)

